# revision 1
# baseline (speedup 1.0000x reference)
"""Trainium2 Bass kernel for nn_ASTDecoder (4-layer transformer decoder,
B=4, S=M=1024, D=512, H=8, DFF=2048, fp32).

Sharding: data-parallel over batch. Core c computes batch element c%4 end to
end (cores 4-7 are duplicates whose outputs are ignored).

On-core layout: activations live feature-major ("transposed", [D, tok]) so
every projection is matmul(lhsT=W[D,out], rhs=actT) with natural weights.
Attention computes transposed scores s^T[k,q] = matmul(lhsT=k^T, rhs=q^T);
softmax skips the max-subtraction (scores are O(1) for this model), the
denominator comes for free from a ones-column appended to V, and the causal
mask is applied as a column-range restriction plus one triangular block
multiply per diagonal tile. All matmuls run in float32r.
"""

import sys

sys.path.insert(0, '/opt/trn_rl_repo')

import ml_dtypes
import numpy as np

import concourse.bass as bass  # noqa: F401
import concourse.tile as tile
import concourse.mybir as mybir
from concourse import bacc
from concourse.bass_utils import run_bass_kernel_spmd

F32 = mybir.dt.float32
F32R = mybir.dt.float32r
BF16 = mybir.dt.bfloat16
AF = mybir.ActivationFunctionType
ALU = mybir.AluOpType

B, S, M, D, H, L, DFF = 4, 1024, 1024, 512, 8, 4, 2048
DK = D // H          # 64
NT = D // 128        # 4 feature tiles
TT = S // 128        # 8 token tiles
QH = S // 512        # 2 query halves

_cache = {}

_tables_patched = False


def _patch_act_tables():
    """Route Exp and Ln to the combined natural_log_exp set so LayerNorm's
    Ln->Exp rstd and the attention exps share one ACT table (avoids ~2.7us
    table reloads inside every LayerNorm critical path)."""
    global _tables_patched
    if _tables_patched:
        return
    import concourse.hw_specs as hw_specs
    orig = hw_specs.get_activation_tables

    def patched(module_arch):
        t = {k: set(v) for k, v in orig(module_arch).items()}
        exp = mybir.ActivationFunctionType.Exp
        ln = mybir.ActivationFunctionType.Ln
        for name, funcs in t.items():
            if name != "natural_log_exp_and_others":
                funcs.discard(exp)
                funcs.discard(ln)
        return t

    hw_specs.get_activation_tables = patched
    bacc.get_activation_tables = patched
    _tables_patched = True


def build_program(num_layers=L):
    _patch_act_tables()
    nc = bacc.Bacc("TRN2", target_bir_lowering=False, debug=False, num_devices=8)
    nl = num_layers

    # ---- DRAM I/O ----
    xT_d = nc.dram_tensor("xT", [D, S], F32R, kind="ExternalInput")
    lmemT_d = nc.dram_tensor("lmemT", [D, M], BF16, kind="ExternalInput")
    rmemT_d = nc.dram_tensor("rmemT", [D, M], BF16, kind="ExternalInput")
    tri_d = nc.dram_tensor("tri", [128, 128], BF16, kind="ExternalInput")

    def wt(name, shape):
        return nc.dram_tensor(name, shape, BF16, kind="ExternalInput")

    def ft(name, shape):
        return nc.dram_tensor(name, shape, F32, kind="ExternalInput")

    w_d = {}
    for a in ("s", "l", "r"):
        for m_ in ("q", "k", "v", "o"):
            w_d[m_ + a] = wt(f"w{m_}{a}", [nl, D, D])
    w_d["1"] = wt("w1", [nl, D, DFF])
    w_d["2"] = wt("w2", [nl, DFF, D])

    b_d = {}
    for a in ("s", "l", "r"):
        for m_ in ("q", "k", "o"):
            b_d[m_ + a] = ft(f"b{m_}{a}", [nl, 128, NT])
        b_d["v" + a] = ft(f"bv{a}", [nl, 1, D])
    b_d["1"] = ft("b1", [nl, 128, DFF // 128])
    b_d["2"] = ft("b2", [nl, 128, NT])
    lns_d = ft("lns", [nl, 4, 128, NT])
    lnb_d = ft("lnb", [nl, 4, 128, NT])
    fns_d = ft("fns", [1, 128, NT])
    fnb_d = ft("fnb", [1, 128, NT])

    out_d = nc.dram_tensor("out", [D, S], F32, kind="ExternalOutput")

    with tile.TileContext(nc) as tc:
        import contextlib
        with contextlib.ExitStack() as ctx:
            big = ctx.enter_context(tc.tile_pool(name="big", bufs=1))
            wpool = ctx.enter_context(tc.tile_pool(name="w", bufs=6))
            epool = ctx.enter_context(tc.tile_pool(name="e", bufs=4))
            scr1 = ctx.enter_context(tc.tile_pool(name="scr1", bufs=1))
            scr2 = ctx.enter_context(tc.tile_pool(name="scr2", bufs=2))
            consts = ctx.enter_context(tc.tile_pool(name="consts", bufs=1))
            lparam = ctx.enter_context(tc.tile_pool(name="lparam", bufs=2))
            ps_mm = ctx.enter_context(tc.tile_pool(name="psmm", bufs=2, space="PSUM"))
            ps_av = ctx.enter_context(tc.tile_pool(name="psav", bufs=2, space="PSUM"))
            ps_sc = ctx.enter_context(tc.tile_pool(name="pssc", bufs=2, space="PSUM"))
            avs_p = ctx.enter_context(tc.tile_pool(name="avs", bufs=4))

            # ---- persistent tiles ----
            xt = big.tile([128, NT, S], F32R, tag="x")
            lmem = big.tile([128, NT, M], BF16, tag="lmem")
            rmem = big.tile([128, NT, M], BF16, tag="rmem")
            # act pool: per-sublayer tiles; same-tag allocations reuse the
            # slot serially (ht and at share "ha": never live simultaneously).
            act = ctx.enter_context(tc.tile_pool(name="act", bufs=1))
            kv = ctx.enter_context(tc.tile_pool(name="kv", bufs=3))

            nc.sync.dma_start(out=xt, in_=xT_d.rearrange("(t p) s -> p t s", p=128))
            nc.sync.dma_start(out=lmem, in_=lmemT_d.rearrange("(t p) s -> p t s", p=128))
            nc.sync.dma_start(out=rmem, in_=rmemT_d.rearrange("(t p) s -> p t s", p=128))

            tri = consts.tile([128, 128], BF16)
            nc.sync.dma_start(out=tri, in_=tri_d[:])
            ones_f = consts.tile([128, 64], F32)
            nc.vector.memset(ones_f, 1.0)
            ones_r = consts.tile([128, 1], F32R)
            nc.vector.tensor_copy(out=ones_r, in_=ones_f[:, 0:1])
            zcol = consts.tile([128, 1], F32)
            nc.vector.memset(zcol, 0.0)
            epst = consts.tile([1, 1], F32)
            nc.vector.memset(epst, 1e-5)

            def load_w(dram_ap):
                t = wpool.tile([128, 4, 512], BF16, tag="w")
                nc.sync.dma_start(out=t, in_=dram_ap)
                return t

            def w_slice(dram, l_, q=None):
                # dram [nl, IN, OUT] -> [128, 4, 512] AP
                a = dram[l_].rearrange("(t p) n -> p t n", p=128)
                if q is not None:  # quarter of the free dim
                    a = a[:, :, q * 512:(q + 1) * 512]
                return a

            def emit_ln(x_in, s_ap, b_ap, out_t, out_dtype_is_f32=False):
                """out_t[:, t, :] = (x - mu)/sqrt(var+eps) * s[t] + b[t].
                s_ap/b_ap: [128, NT] SBUF APs."""
                for qh in range(QH):
                    sl = slice(qh * 512, (qh + 1) * 512)
                    sum1 = ps_mm.tile([1, 512], F32, tag="mm", name="sum1")
                    sum2 = ps_mm.tile([1, 512], F32, tag="mm", name="sum2")
                    for k in range(NT):
                        nc.tensor.matmul(sum1, ones_r, x_in[:, k, sl],
                                         start=(k == 0), stop=(k == NT - 1))
                    for k in range(NT):
                        sqt = scr2.tile([128, 512], F32R, tag="sq")
                        nc.vector.tensor_mul(sqt, x_in[:, k, sl], x_in[:, k, sl])
                        nc.tensor.matmul(sum2, ones_r, sqt,
                                         start=(k == 0), stop=(k == NT - 1))
                    mu = scr1.tile([1, 512], F32, tag="mu")
                    nc.vector.tensor_scalar_mul(mu, sum1, 1.0 / D)
                    mm = scr1.tile([1, 512], F32, tag="mm2")
                    nc.vector.tensor_mul(mm, mu, mu)
                    var = scr1.tile([1, 512], F32, tag="var")
                    nc.vector.scalar_tensor_tensor(
                        out=var, in0=sum2, scalar=1.0 / D, in1=mm,
                        op0=ALU.mult, op1=ALU.subtract)
                    rstd = scr1.tile([1, 512], F32, tag="rstd")
                    nc.scalar.activation(rstd, var, AF.Ln, bias=epst)
                    nc.scalar.activation(rstd, rstd, AF.Exp, scale=-0.5)
                    mub = scr1.tile([128, 512], F32, tag="mub")
                    rstdb = scr1.tile([128, 512], F32, tag="rstdb")
                    nc.gpsimd.partition_broadcast(mub, mu)
                    nc.gpsimd.partition_broadcast(rstdb, rstd)
                    for k in range(NT):
                        tmp = scr1.tile([128, 512], F32, tag="lntmp")
                        nc.vector.tensor_sub(tmp, x_in[:, k, sl], mub)
                        nc.vector.tensor_mul(tmp, tmp, rstdb)
                        nc.vector.tensor_scalar(
                            out=out_t[:, k, sl], in0=tmp,
                            scalar1=s_ap[:, k:k + 1], scalar2=b_ap[:, k:k + 1],
                            op0=ALU.mult, op1=ALU.add)

            def emit_projT(w_sb, rhs_t, out_t, bias_sb):
                """out_t[:, m, :] (f32r, [128,NT,S]) = W^T @ rhs + bias.
                w_sb [128,4,512], rhs_t [128,NT,S] f32r, bias_sb [128,NT]."""
                for m_ in range(NT):
                    for qh in range(QH):
                        sl = slice(qh * 512, (qh + 1) * 512)
                        ps = ps_mm.tile([128, 512], F32, tag="mm")
                        for k in range(NT):
                            nc.tensor.matmul(
                                ps, w_sb[:, k, m_ * 128:(m_ + 1) * 128],
                                rhs_t[:, k, sl],
                                start=(k == 0), stop=(k == NT - 1))
                        nc.vector.tensor_scalar(
                            out=out_t[:, m_, sl], in0=ps,
                            scalar1=bias_sb[:, m_:m_ + 1], scalar2=None,
                            op0=ALU.add)

            def emit_v(w_sb, src_t, bvb, vt):
                """vt[:, mt, h, 0:DK] = (src^T)^T @ Wv + bv (natural layout)."""
                nc.vector.tensor_copy(
                    out=vt[:, :, :, DK:DK + 1].rearrange("p a b c -> p (a b c)"),
                    in_=ones_f)
                for mt in range(TT):
                    ps = ps_mm.tile([128, 512], F32, tag="mm")
                    for k in range(NT):
                        nc.tensor.matmul(
                            ps, src_t[:, k, mt * 128:(mt + 1) * 128],
                            w_sb[:, k, :],
                            start=(k == 0), stop=(k == NT - 1))
                    nc.vector.tensor_tensor(
                        out=vt[:, mt, :, 0:DK],
                        in0=ps.rearrange("p (h d) -> p h d", h=H),
                        in1=bvb.rearrange("p (h d) -> p h d", h=H),
                        op=ALU.add)

            def emit_attn(is_self, qt, kt_t, vt, at):
                """at = softmax(k^T q / sqrt(dk)) V, all transposed layouts.
                Heads run in even/odd pairs: their score matmuls use PE row
                groups 0-63 / 64-127 (tile_position auto-derived from the
                lhsT base partition) and execute concurrently."""
                for qh in range(QH):
                    ktmax = (qh + 1) * 4 if is_self else TT
                    for hp in range(H // 2):
                        ets = []
                        for sub in range(2):
                            ets.append(epool.tile([128, TT, 512], BF16, tag="e",
                                                  name=f"et{sub}"))
                        # interleaved head-pair scores; each sc tile holds
                        # 2 k-tiles (2 PSUM banks) so one exp covers both
                        for kg in range((ktmax + 1) // 2):
                            kts = [k for k in (2 * kg, 2 * kg + 1) if k < ktmax]
                            scs = [ps_sc.tile([128, 2, 512], F32, tag="sc",
                                              name=f"sc{s_}") for s_ in range(2)]
                            for j, k in enumerate(kts):
                                c0 = max(0, k - qh * 4) * 128 if is_self else 0
                                for sub in range(2):
                                    h_ = 2 * hp + sub
                                    po = (h_ % 2) * 64
                                    ft_ = h_ // 2
                                    nc.tensor.matmul(
                                        scs[sub][:, j, c0:],
                                        kt_t[po:po + 64, ft_, k * 128:(k + 1) * 128],
                                        qt[po:po + 64, ft_, qh * 512 + c0:(qh + 1) * 512],
                                        start=True, stop=True)
                            cmin = (max(0, 2 * kg - qh * 4) * 128
                                    if is_self else 0)
                            for sub in range(2):
                                nc.scalar.activation(
                                    ets[sub][:, 2 * kg:2 * kg + len(kts), cmin:],
                                    scs[sub][:, 0:len(kts), cmin:], AF.Exp,
                                    scale=1.0 / np.sqrt(DK))
                                if is_self:
                                    for k in kts:
                                        if k >= qh * 4:
                                            c0 = (k - qh * 4) * 128
                                            nc.vector.tensor_mul(
                                                ets[sub][:, k, c0:c0 + 128],
                                                ets[sub][:, k, c0:c0 + 128], tri)
                        for sub in range(2):
                            h_ = 2 * hp + sub
                            po = (h_ % 2) * 64
                            ft_ = h_ // 2
                            et = ets[sub]
                            av = ps_av.tile([DK + 1, 512], F32, tag="av")
                            for k in range(ktmax):
                                c0 = max(0, k - qh * 4) * 128 if is_self else 0
                                nc.tensor.matmul(
                                    av[:, c0:], vt[:, k, h_, :], et[:, k, c0:],
                                    start=(k == 0), stop=(k == ktmax - 1))
                            sb65 = avs_p.tile([DK + 1, 512], F32, tag="avs")
                            nc.vector.tensor_copy(out=sb65, in_=av)
                            rds = scr1.tile([1, 512], F32, tag="rds")
                            nc.vector.tensor_copy(out=rds, in_=sb65[DK:DK + 1, :])
                            rd = scr1.tile([1, 512], F32, tag="rd")
                            nc.vector.reciprocal_approx_fast(out=rd, in_=rds)
                            rdb = scr2.tile([64, 512], F32, tag="rdb")
                            nc.gpsimd.partition_broadcast(rdb, rd)
                            nc.vector.tensor_mul(
                                at[po:po + 64, ft_, qh * 512:(qh + 1) * 512],
                                sb65[0:DK, :], rdb)

            def emit_resid(w_sb, rhs_t, bias_sb):
                """x += W^T @ rhs + bias (out-projection / FFN-2 path)."""
                for m_ in range(NT):
                    for qh in range(QH):
                        sl = slice(qh * 512, (qh + 1) * 512)
                        ps = ps_mm.tile([128, 512], F32, tag="mm")
                        for k in range(NT):
                            nc.tensor.matmul(
                                ps, w_sb[:, k, m_ * 128:(m_ + 1) * 128],
                                rhs_t[:, k, sl],
                                start=(k == 0), stop=(k == NT - 1))
                        nc.vector.scalar_tensor_tensor(
                            out=xt[:, m_, sl], in0=ps,
                            scalar=bias_sb[:, m_:m_ + 1], in1=xt[:, m_, sl],
                            op0=ALU.add, op1=ALU.add)

            def load_bias_pp(dram, l_):
                t = lparam.tile([128, NT], F32, tag="bpp")
                nc.sync.dma_start(out=t, in_=dram[l_])
                return t

            for l_ in range(nl):
                lns = lparam.tile([128, 4, NT], F32, tag="lns")
                lnb = lparam.tile([128, 4, NT], F32, tag="lnb")
                nc.sync.dma_start(out=lns, in_=lns_d[l_].rearrange("a p t -> p a t"))
                nc.sync.dma_start(out=lnb, in_=lnb_d[l_].rearrange("a p t -> p a t"))

                # -- self QKV --
                ht = act.tile([128, NT, S], BF16, tag="ha")
                emit_ln(xt, lns[:, 0, :], lnb[:, 0, :], ht)
                kvt = {}

                def qkv(a, src_t, with_q):
                    wk = load_w(w_slice(w_d["k" + a], l_))
                    wv = load_w(w_slice(w_d["v" + a], l_))
                    bk = load_bias_pp(b_d["k" + a], l_)
                    bvr = scr1.tile([1, D], F32, tag="bvr")
                    nc.sync.dma_start(out=bvr, in_=b_d["v" + a][l_])
                    bvb = scr1.tile([128, D], F32, tag="bvb")
                    nc.gpsimd.partition_broadcast(bvb, bvr)
                    kt_t = kv.tile([128, NT, S], BF16, tag="kt")
                    vt = kv.tile([128, TT, H, DK + 1], BF16, tag="vt")
                    emit_projT(wk, src_t, kt_t, bk)
                    emit_v(wv, src_t, bvb, vt)
                    kvt[a] = (kt_t, vt)

                def qproj(a, ht_):
                    wq = load_w(w_slice(w_d["q" + a], l_))
                    bq = load_bias_pp(b_d["q" + a], l_)
                    qt = act.tile([128, NT, S], BF16, tag="qt")
                    emit_projT(wq, ht_, qt, bq)
                    return qt

                def oproj(a, at_):
                    wo = load_w(w_slice(w_d["o" + a], l_))
                    bo = load_bias_pp(b_d["o" + a], l_)
                    emit_resid(wo, at_, bo)

                qt = qproj("s", ht)
                qkv("s", ht, True)
                at = act.tile([128, NT, S], BF16, tag="ha")
                emit_attn(True, qt, kvt["s"][0], kvt["s"][1], at)
                oproj("s", at)
                # cross K/V: no dependency on x -- emitted after self-attn so
                # the scheduler uses them to fill PE gaps (LN chains etc.)
                qkv("l", lmem, False)
                qkv("r", rmem, False)

                for si, a in ((1, "l"), (2, "r")):
                    ht = act.tile([128, NT, S], BF16, tag="ha")
                    emit_ln(xt, lns[:, si, :], lnb[:, si, :], ht)
                    qt = qproj(a, ht)
                    at = act.tile([128, NT, S], BF16, tag="ha")
                    emit_attn(False, qt, kvt[a][0], kvt[a][1], at)
                    oproj(a, at)

                # ---- FFN ----
                ht = act.tile([128, NT, S], BF16, tag="ha")
                emit_ln(xt, lns[:, 3, :], lnb[:, 3, :], ht)
                b1 = lparam.tile([128, DFF // 128], F32, tag="b1")
                nc.sync.dma_start(out=b1, in_=b_d["1"][l_])
                b2 = load_bias_pp(b_d["2"], l_)
                for qr in range(4):
                    h1 = act.tile([128, 4, S], BF16, tag="h1")
                    w1 = load_w(w_slice(w_d["1"], l_, q=qr))
                    w2 = load_w(
                        w_d["2"][l_].rearrange("(t p) n -> p t n", p=128)
                        [:, qr * 4:(qr + 1) * 4, :])
                    for dt_ in range(4):
                        for qh in range(QH):
                            sl = slice(qh * 512, (qh + 1) * 512)
                            ps = ps_mm.tile([128, 512], F32, tag="mm")
                            for k in range(NT):
                                nc.tensor.matmul(
                                    ps, w1[:, k, dt_ * 128:(dt_ + 1) * 128],
                                    ht[:, k, sl],
                                    start=(k == 0), stop=(k == NT - 1))
                            nc.scalar.activation(
                                h1[:, dt_, sl], ps, AF.Gelu_apprx_tanh,
                                bias=b1[:, qr * 4 + dt_:qr * 4 + dt_ + 1])
                    for m_ in range(NT):
                        for qh in range(QH):
                            sl = slice(qh * 512, (qh + 1) * 512)
                            ps = ps_mm.tile([128, 512], F32, tag="mm")
                            for dt_ in range(4):
                                nc.tensor.matmul(
                                    ps, w2[:, dt_, m_ * 128:(m_ + 1) * 128],
                                    h1[:, dt_, sl],
                                    start=(dt_ == 0), stop=(dt_ == 3))
                            bsl = b2[:, m_:m_ + 1] if qr == 0 else zcol
                            nc.vector.scalar_tensor_tensor(
                                out=xt[:, m_, sl], in0=ps, scalar=bsl,
                                in1=xt[:, m_, sl], op0=ALU.add, op1=ALU.add)

            # ---- final LN + output ----
            fns = lparam.tile([128, NT], F32, tag="fns")
            fnb = lparam.tile([128, NT], F32, tag="fnb")
            nc.sync.dma_start(out=fns, in_=fns_d[0])
            nc.sync.dma_start(out=fnb, in_=fnb_d[0])
            outt = act.tile([128, NT, S], F32, tag="qt")
            emit_ln(xt, fns, fnb, outt, out_dtype_is_f32=True)
            nc.sync.dma_start(out=out_d.rearrange("(t p) s -> p t s", p=128),
                              in_=outt)

    nc.compile()
    return nc


def _prep_inputs(inputs, num_layers=L):
    """Build per-core in_maps from the full problem inputs."""
    nl = num_layers
    f32 = np.float32
    g = {k: np.asarray(v, dtype=f32) if np.asarray(v).dtype != np.bool_ else v
         for k, v in inputs.items()}

    def pp(a):  # [nl, D] -> [nl, 128, NT] per-partition layout
        return np.ascontiguousarray(
            a[:nl].reshape(nl, NT, 128).transpose(0, 2, 1))

    tri = np.tril(np.ones((128, 128), f32)).T  # tri[p, j] = 1 if p <= j
    common = {}
    for i, a in enumerate(("s", "l", "r")):
        wqkv = g["Wqkv_self" if a == "s" else f"Wqkv_{a}"][:nl]
        bqkv = g["bqkv_self" if a == "s" else f"bqkv_{a}"][:nl]
        wo = g["Wo_self" if a == "s" else f"Wo_{a}"][:nl]
        bo = g["bo_self" if a == "s" else f"bo_{a}"][:nl]
        common[f"wq{a}"] = np.ascontiguousarray(wqkv[:, 0]).astype(ml_dtypes.bfloat16)
        common[f"wk{a}"] = np.ascontiguousarray(wqkv[:, 1]).astype(ml_dtypes.bfloat16)
        common[f"wv{a}"] = np.ascontiguousarray(wqkv[:, 2]).astype(ml_dtypes.bfloat16)
        common[f"wo{a}"] = np.ascontiguousarray(wo).astype(ml_dtypes.bfloat16)
        common[f"bq{a}"] = pp(bqkv[:, 0])
        common[f"bk{a}"] = pp(bqkv[:, 1])
        common[f"bv{a}"] = np.ascontiguousarray(bqkv[:, 2]).reshape(nl, 1, D)
        common[f"bo{a}"] = pp(bo)
    common["w1"] = np.ascontiguousarray(g["W1"][:nl]).astype(ml_dtypes.bfloat16)
    common["w2"] = np.ascontiguousarray(g["W2"][:nl]).astype(ml_dtypes.bfloat16)
    common["b1"] = np.ascontiguousarray(
        g["b1"][:nl].reshape(nl, DFF // 128, 128).transpose(0, 2, 1))
    common["b2"] = pp(g["b2"][:nl])
    common["lns"] = np.ascontiguousarray(
        g["ln_scale"][:nl].reshape(nl, 4, NT, 128).transpose(0, 1, 3, 2))
    common["lnb"] = np.ascontiguousarray(
        g["ln_bias"][:nl].reshape(nl, 4, NT, 128).transpose(0, 1, 3, 2))
    common["fns"] = g["fnorm_scale"].reshape(1, NT, 128).transpose(0, 2, 1).copy()
    common["fnb"] = g["fnorm_bias"].reshape(1, NT, 128).transpose(0, 2, 1).copy()
    common["tri"] = tri.astype(ml_dtypes.bfloat16)

    in_maps = []
    for c in range(8):
        b = c % B
        m = dict(common)
        m["xT"] = np.ascontiguousarray(g["tgt_emb"][b].T)
        m["lmemT"] = np.ascontiguousarray(g["l_mem_emb"][b].T).astype(ml_dtypes.bfloat16)
        m["rmemT"] = np.ascontiguousarray(g["r_mem_emb"][b].T).astype(ml_dtypes.bfloat16)
        in_maps.append(m)
    return in_maps


def run(inputs, num_layers=L, trace=False, tmpdir=None):
    key = num_layers
    if key not in _cache:
        _cache[key] = build_program(num_layers)
    nc = _cache[key]
    in_maps = _prep_inputs(inputs, num_layers)
    res = run_bass_kernel_spmd(nc, in_maps, core_ids=list(range(8)),
                               trace=trace, tmpdir=tmpdir)
    out = np.stack([res.results[b]["out"].T for b in range(B)])
    return out, res


def kernel(**inputs):
    out, _ = run(inputs)
    return out.astype(np.float32)



# revision 15
# speedup vs baseline: 1.5507x; 1.5507x over previous
"""Trainium2 Bass kernel for nn_ASTDecoder (4-layer transformer decoder,
B=4, S=M=1024, D=512, H=8, DFF=2048, fp32).

Sharding: 4-way data-parallel over batch x 2-way sequence-parallel within
each core pair. Core c handles batch c//2; rank r = c%2 owns tokens r::2
(parity interleave keeps the causal-attention work balanced while giving
both ranks an identical instruction stream -- only mask DATA differs).

Per layer, each core LNs/projects only its 512 tokens. Self-attention K/V
shards are exchanged with a pairwise AllGather (HBM->HBM collective); the
gathered KV is laid out [even-token tiles | odd-token tiles] and causality
is enforced with two per-rank input masks (tri / strict-tri) applied after
the exp. Cross-attention K/V are computed redundantly from the full static
memory inputs (cheaper than 8 more collectives).

On-core layout: activations live feature-major ("transposed", [D, tok]) so
every projection is matmul(lhsT=W[D,out], rhs=actT) with natural weights.
Attention computes transposed scores s^T[k,q] = matmul(lhsT=k^T, rhs=q^T);
softmax skips the max-subtraction (scores are O(1) for this model), the
denominator comes for free from a ones-column appended to V.
"""

import sys

sys.path.insert(0, '/opt/trn_rl_repo')

import ml_dtypes
import numpy as np

import concourse.bass as bass  # noqa: F401
import concourse.tile as tile
import concourse.mybir as mybir
from concourse import bacc
from concourse.bass_utils import run_bass_kernel_spmd

F32 = mybir.dt.float32
F32R = mybir.dt.float32r
BF16 = mybir.dt.bfloat16
AF = mybir.ActivationFunctionType
ALU = mybir.AluOpType

B, S, M, D, H, L, DFF = 4, 1024, 1024, 512, 8, 4, 2048
DK = D // H          # 64
NT = D // 128        # 4 feature tiles
SQ = S // 2          # 512 tokens per core (sequence-parallel)
TT = M // 128        # 8 kv token tiles (gathered self / full cross)
MQ = M // 512        # 2 slabs for cross-memory K/V projections
PAIRS = [[0, 1], [2, 3], [4, 5], [6, 7]]

_cache = {}

_tables_patched = False


def _patch_act_tables():
    """Route Exp and Ln to the combined natural_log_exp set so LayerNorm's
    Ln->Exp rstd and the attention exps share one ACT table (avoids ~2.7us
    table reloads inside every LayerNorm critical path)."""
    global _tables_patched
    if _tables_patched:
        return
    import concourse.hw_specs as hw_specs
    orig = hw_specs.get_activation_tables

    def patched(module_arch):
        t = {k: set(v) for k, v in orig(module_arch).items()}
        exp = mybir.ActivationFunctionType.Exp
        ln = mybir.ActivationFunctionType.Ln
        for name, funcs in t.items():
            if name != "natural_log_exp_and_others":
                funcs.discard(exp)
                funcs.discard(ln)
        return t

    hw_specs.get_activation_tables = patched
    bacc.get_activation_tables = patched
    _tables_patched = True


def build_program(num_layers=L):
    _patch_act_tables()
    nc = bacc.Bacc("TRN2", target_bir_lowering=False, debug=False, num_devices=8)
    nl = num_layers

    # ---- DRAM I/O ----
    xT_d = nc.dram_tensor("xT", [D, SQ], F32R, kind="ExternalInput")
    lmemT_d = nc.dram_tensor("lmemT", [D, M], BF16, kind="ExternalInput")
    rmemT_d = nc.dram_tensor("rmemT", [D, M], BF16, kind="ExternalInput")
    me_d = nc.dram_tensor("m_e", [128, 128], BF16, kind="ExternalInput")
    mo_d = nc.dram_tensor("m_o", [128, 128], BF16, kind="ExternalInput")

    def wt(name, shape):
        return nc.dram_tensor(name, shape, BF16, kind="ExternalInput")

    def ft(name, shape):
        return nc.dram_tensor(name, shape, F32, kind="ExternalInput")

    w_d = {}
    for a in ("s", "l", "r"):
        for m_ in ("q", "k", "v", "o"):
            w_d[m_ + a] = wt(f"w{m_}{a}", [nl, D, D])
    w_d["1"] = wt("w1", [nl, D, DFF])
    w_d["2"] = wt("w2", [nl, DFF, D])

    b_d = {}
    for a in ("s", "l", "r"):
        for m_ in ("q", "k", "o"):
            b_d[m_ + a] = ft(f"b{m_}{a}", [nl, 128, NT])
        b_d["v" + a] = ft(f"bv{a}", [nl, 1, D])
    b_d["1"] = ft("b1", [nl, 128, DFF // 128])
    b_d["2"] = ft("b2", [nl, 128, NT])
    lns_d = ft("lns", [nl, 4, 128, NT])
    lnb_d = ft("lnb", [nl, 4, 128, NT])
    fns_d = ft("fns", [1, 128, NT])
    fnb_d = ft("fnb", [1, 128, NT])

    out_d = nc.dram_tensor("out", [D, SQ], F32, kind="ExternalOutput")

    import os
    dbg = os.environ.get("KDBG", "") == "1"
    dbg_d = {}
    if dbg:
        dbg_d["ht"] = nc.dram_tensor("dbg_ht", [128, NT, SQ], BF16,
                                     kind="ExternalOutput")
        dbg_d["kt"] = nc.dram_tensor("dbg_kt", [128, NT, 2 * SQ], BF16,
                                     kind="ExternalOutput")
        dbg_d["vt"] = nc.dram_tensor("dbg_vt", [128, TT, H, DK + 1], BF16,
                                     kind="ExternalOutput")
        dbg_d["qt"] = nc.dram_tensor("dbg_qt", [128, NT, SQ], BF16,
                                     kind="ExternalOutput")
        dbg_d["at"] = nc.dram_tensor("dbg_at", [128, NT, SQ], BF16,
                                     kind="ExternalOutput")
        dbg_d["x1"] = nc.dram_tensor("dbg_x1", [128, NT, SQ], F32,
                                     kind="ExternalOutput")
        dbg_d["dn0"] = nc.dram_tensor("dbg_dn0", [1, SQ], F32,
                                      kind="ExternalOutput")
        dbg_d["nm0"] = nc.dram_tensor("dbg_nm0", [64, SQ], F32,
                                      kind="ExternalOutput")
        for nm_ in ("x2", "x3", "x4", "atl", "atr", "ktl", "vtl", "qtl"):
            shp = [128, NT, SQ]
            if nm_ == "ktl":
                shp = [128, NT, M]
            if nm_ == "vtl":
                shp = [128, TT, H, DK + 1]
            dbg_d[nm_] = nc.dram_tensor(
                f"dbg_{nm_}", shp, F32 if nm_[0] == "x" else BF16,
                kind="ExternalOutput")

    # KV exchange bounce buffers (K: NT*SQ cols, V: 4*H*(DK+1) cols)
    KC = NT * SQ                  # 2048
    VC = 4 * H * (DK + 1)         # 2080
    cc_in_d = [nc.dram_tensor(f"ccin{l_}", [128, KC + VC], BF16, kind="Internal")
               for l_ in range(nl)]
    cc_out_d = [nc.dram_tensor(f"ccout{l_}", [2, 128, KC + VC], BF16,
                               kind="Internal")
                for l_ in range(nl)]

    with tile.TileContext(nc) as tc:
        import contextlib
        with contextlib.ExitStack() as ctx:
            big = ctx.enter_context(tc.tile_pool(name="big", bufs=1))
            wpool = ctx.enter_context(tc.tile_pool(name="w", bufs=6))
            epool = ctx.enter_context(tc.tile_pool(name="e", bufs=4))
            scr1 = ctx.enter_context(tc.tile_pool(name="scr1", bufs=1))
            scr2 = ctx.enter_context(tc.tile_pool(name="scr2", bufs=2))
            consts = ctx.enter_context(tc.tile_pool(name="consts", bufs=1))
            lparam = ctx.enter_context(tc.tile_pool(name="lparam", bufs=2))
            ps_mm = ctx.enter_context(tc.tile_pool(name="psmm", bufs=2, space="PSUM"))
            ps_av = ctx.enter_context(tc.tile_pool(name="psav", bufs=2, space="PSUM"))
            ps_sc = ctx.enter_context(tc.tile_pool(name="pssc", bufs=2, space="PSUM"))

            # ---- persistent tiles ----
            xt = big.tile([128, NT, SQ], F32R, tag="x")
            lmem = big.tile([128, NT, M], BF16, tag="lmem")
            rmem = big.tile([128, NT, M], BF16, tag="rmem")
            act = ctx.enter_context(tc.tile_pool(name="act", bufs=1))
            kv = ctx.enter_context(tc.tile_pool(name="kv", bufs=3))
            kvl = ctx.enter_context(tc.tile_pool(name="kvl", bufs=2))

            nc.sync.dma_start(out=xt, in_=xT_d.rearrange("(t p) s -> p t s", p=128))
            nc.sync.dma_start(out=lmem, in_=lmemT_d.rearrange("(t p) s -> p t s", p=128))
            nc.sync.dma_start(out=rmem, in_=rmemT_d.rearrange("(t p) s -> p t s", p=128))

            m_e = consts.tile([128, 128], BF16)
            nc.sync.dma_start(out=m_e, in_=me_d[:])
            m_o = consts.tile([128, 128], BF16)
            nc.sync.dma_start(out=m_o, in_=mo_d[:])
            ones_f = consts.tile([128, 64], F32)
            nc.vector.memset(ones_f, 1.0)
            ones_r = consts.tile([128, 1], F32R)
            nc.vector.tensor_copy(out=ones_r, in_=ones_f[:, 0:1])
            zcol = consts.tile([128, 1], F32)
            nc.vector.memset(zcol, 0.0)
            epst = consts.tile([1, 1], F32)
            nc.vector.memset(epst, 1e-5)

            def load_w(dram_ap):
                t = wpool.tile([128, 4, 512], BF16, tag="w")
                nc.sync.dma_start(out=t, in_=dram_ap)
                return t

            def w_slice(dram, l_, q=None):
                # dram [nl, IN, OUT] -> [128, 4, 512] AP
                a = dram[l_].rearrange("(t p) n -> p t n", p=128)
                if q is not None:  # quarter of the free dim
                    a = a[:, :, q * 512:(q + 1) * 512]
                return a

            def emit_ln(x_in, s_ap, b_ap, out_t):
                """out_t[:, t, :] = (x - mu)/sqrt(var+eps) * s[t] + b[t].
                s_ap/b_ap: [128, NT] SBUF APs. x_in/out_t: [128, NT, SQ]."""
                sum1 = ps_mm.tile([1, SQ], F32, tag="mm", name="sum1")
                sum2 = ps_mm.tile([1, SQ], F32, tag="mm", name="sum2")
                for k in range(NT):
                    nc.tensor.matmul(sum1, ones_r, x_in[:, k, :],
                                     start=(k == 0), stop=(k == NT - 1))
                for k in range(NT):
                    sqt = scr2.tile([128, SQ], F32R, tag="sq")
                    nc.vector.tensor_mul(sqt, x_in[:, k, :], x_in[:, k, :])
                    nc.tensor.matmul(sum2, ones_r, sqt,
                                     start=(k == 0), stop=(k == NT - 1))
                mu = scr1.tile([1, SQ], F32, tag="mu")
                nc.vector.tensor_scalar_mul(mu, sum1, 1.0 / D)
                mm = scr1.tile([1, SQ], F32, tag="mm2")
                nc.vector.tensor_mul(mm, mu, mu)
                var = scr1.tile([1, SQ], F32, tag="var")
                nc.vector.scalar_tensor_tensor(
                    out=var, in0=sum2, scalar=1.0 / D, in1=mm,
                    op0=ALU.mult, op1=ALU.subtract)
                rstd = scr1.tile([1, SQ], F32, tag="rstd")
                nc.scalar.activation(rstd, var, AF.Ln, bias=epst)
                nc.scalar.activation(rstd, rstd, AF.Exp, scale=-0.5)
                mub = scr1.tile([128, SQ], F32, tag="mub")
                rstdb = scr1.tile([128, SQ], F32, tag="rstdb")
                nc.gpsimd.partition_broadcast(mub, mu)
                nc.gpsimd.partition_broadcast(rstdb, rstd)
                for k in range(NT):
                    tmp = scr1.tile([128, SQ], F32, tag="lntmp")
                    nc.vector.tensor_sub(tmp, x_in[:, k, :], mub)
                    nc.vector.tensor_mul(tmp, tmp, rstdb)
                    nc.vector.tensor_scalar(
                        out=out_t[:, k, :], in0=tmp,
                        scalar1=s_ap[:, k:k + 1], scalar2=b_ap[:, k:k + 1],
                        op0=ALU.mult, op1=ALU.add)

            def emit_projT(w_sb, rhs_t, out_t, bias_sb, ncols=SQ):
                """out_t[:, m, :] ([128,NT,ncols]) = W^T @ rhs + bias.
                w_sb [128,4,512], rhs_t [128,NT,ncols], bias_sb [128,NT]."""
                for m_ in range(NT):
                    for sl0 in range(0, ncols, 512):
                        sl = slice(sl0, sl0 + 512)
                        ps = ps_mm.tile([128, 512], F32, tag="mm")
                        for k in range(NT):
                            nc.tensor.matmul(
                                ps, w_sb[:, k, m_ * 128:(m_ + 1) * 128],
                                rhs_t[:, k, sl],
                                start=(k == 0), stop=(k == NT - 1))
                        nc.vector.tensor_scalar(
                            out=out_t[:, m_, sl], in0=ps,
                            scalar1=bias_sb[:, m_:m_ + 1], scalar2=None,
                            op0=ALU.add)

            def emit_v(w_sb, src_t, bvb, vt, ntiles):
                """vt[:, mt, h, 0:DK] = (src^T)^T @ Wv + bv (natural layout)."""
                nc.vector.tensor_copy(
                    out=vt[:, :, :, DK:DK + 1].rearrange("p a b c -> p (a b c)"),
                    in_=ones_f[:, 0:ntiles * H])
                for mt in range(ntiles):
                    ps = ps_mm.tile([128, 512], F32, tag="mm")
                    for k in range(NT):
                        nc.tensor.matmul(
                            ps, src_t[:, k, mt * 128:(mt + 1) * 128],
                            w_sb[:, k, :],
                            start=(k == 0), stop=(k == NT - 1))
                    nc.vector.tensor_tensor(
                        out=vt[:, mt, :, 0:DK],
                        in0=ps.rearrange("p (h d) -> p h d", h=H),
                        in1=bvb.rearrange("p (h d) -> p h d", h=H),
                        op=ALU.add)

            def emit_attn(is_self, qt, kt_t, vt, at, dbg_attn=False):
                """at = softmax(k^T q / sqrt(dk)) V, all transposed layouts.
                kt_t [128, NT, 2*SQ] (self: rank-major even|odd tiles) or
                [128, NT, M] (cross). vt [128, TT, H, DK+1]. Heads run in
                even/odd pairs on PE row groups 0-63 / 64-127."""
                for hp in range(H // 2):
                    ets = []
                    for sub in range(2):
                        ets.append(epool.tile([128, TT, SQ], BF16, tag="e",
                                              name=f"et{sub}"))
                    for g in range(4):
                        c0 = g * 128 if is_self else 0
                        # gathered kv tile index for slot (g, jj)
                        js = (g, 4 + g) if is_self else (2 * g, 2 * g + 1)
                        scs = [ps_sc.tile([128, 2, SQ], F32, tag="sc",
                                          name=f"sc{s_}") for s_ in range(2)]
                        for jj, j in enumerate(js):
                            for sub in range(2):
                                h_ = 2 * hp + sub
                                po = (h_ % 2) * 64
                                ft_ = h_ // 2
                                nc.tensor.matmul(
                                    scs[sub][:, jj, c0:],
                                    kt_t[po:po + 64, ft_, j * 128:(j + 1) * 128],
                                    qt[po:po + 64, ft_, c0:],
                                    start=True, stop=True)
                        for sub in range(2):
                            nc.scalar.activation(
                                ets[sub][:, 2 * g:2 * g + 2, c0:],
                                scs[sub][:, 0:2, c0:], AF.Exp,
                                scale=1.0 / np.sqrt(DK))
                            if is_self:
                                nc.vector.tensor_mul(
                                    ets[sub][:, 2 * g, c0:c0 + 128],
                                    ets[sub][:, 2 * g, c0:c0 + 128], m_e)
                                nc.vector.tensor_mul(
                                    ets[sub][:, 2 * g + 1, c0:c0 + 128],
                                    ets[sub][:, 2 * g + 1, c0:c0 + 128], m_o)
                    for sub in range(2):
                        h_ = 2 * hp + sub
                        po = (h_ % 2) * 64
                        ft_ = h_ // 2
                        et = ets[sub]
                        av = ps_av.tile([DK + 1, SQ], F32, tag="av")
                        for i in range(TT):
                            g, jj = i // 2, i % 2
                            c0 = g * 128 if is_self else 0
                            j = (g if jj == 0 else 4 + g) if is_self else i
                            nc.tensor.matmul(
                                av[:, c0:], vt[:, j, h_, :], et[:, i, c0:],
                                start=(i == 0), stop=(i == TT - 1))
                        rds = scr1.tile([1, SQ], F32, tag="rds")
                        nc.vector.tensor_copy(out=rds, in_=av[DK:DK + 1, :])
                        rd = scr1.tile([1, SQ], F32, tag="rd")
                        nc.vector.reciprocal_approx_fast(out=rd, in_=rds)
                        rdb = scr2.tile([64, SQ], F32, tag="rdb")
                        nc.gpsimd.partition_broadcast(rdb, rd)
                        nc.vector.tensor_mul(
                            at[po:po + 64, ft_, :], av[0:DK, :], rdb)
                        if dbg_attn and hp == 0 and sub == 0:
                            nc.sync.dma_start(out=dbg_d["dn0"][:], in_=rds)
                            nm = scr2.tile([64, SQ], F32, tag="nm")
                            nc.vector.tensor_copy(out=nm, in_=av[0:DK, :])
                            nc.sync.dma_start(out=dbg_d["nm0"][:], in_=nm)

            def emit_resid(w_sb, rhs_t, bias_sb):
                """x += W^T @ rhs + bias (out-projection / FFN-2 path)."""
                for m_ in range(NT):
                    ps = ps_mm.tile([128, 512], F32, tag="mm")
                    for k in range(NT):
                        nc.tensor.matmul(
                            ps, w_sb[:, k, m_ * 128:(m_ + 1) * 128],
                            rhs_t[:, k, :],
                            start=(k == 0), stop=(k == NT - 1))
                    nc.vector.scalar_tensor_tensor(
                        out=xt[:, m_, :], in0=ps,
                        scalar=bias_sb[:, m_:m_ + 1], in1=xt[:, m_, :],
                        op0=ALU.add, op1=ALU.add)

            def load_bias_pp(dram, l_):
                t = lparam.tile([128, NT], F32, tag="bpp")
                nc.sync.dma_start(out=t, in_=dram[l_])
                return t

            def load_bvb(a, l_):
                bvr = scr1.tile([1, D], F32, tag="bvr")
                nc.sync.dma_start(out=bvr, in_=b_d["v" + a][l_])
                bvb = scr1.tile([128, D], F32, tag="bvb")
                nc.gpsimd.partition_broadcast(bvb, bvr)
                return bvb

            for l_ in range(nl):
                lns = lparam.tile([128, 4, NT], F32, tag="lns")
                lnb = lparam.tile([128, 4, NT], F32, tag="lnb")
                nc.sync.dma_start(out=lns, in_=lns_d[l_].rearrange("a p t -> p a t"))
                nc.sync.dma_start(out=lnb, in_=lnb_d[l_].rearrange("a p t -> p a t"))

                # -- self LN + local K/V shard --
                ht = act.tile([128, NT, SQ], BF16, tag="ha")
                emit_ln(xt, lns[:, 0, :], lnb[:, 0, :], ht)
                if dbg and l_ == 0:
                    nc.sync.dma_start(out=dbg_d["ht"][:], in_=ht)

                wk = load_w(w_slice(w_d["ks"], l_))
                wv = load_w(w_slice(w_d["vs"], l_))
                bk = load_bias_pp(b_d["ks"], l_)
                bvb = load_bvb("s", l_)
                kt_loc = kvl.tile([128, NT, SQ], BF16, tag="ktl")
                vt_loc = kvl.tile([128, 4, H, DK + 1], BF16, tag="vtl")
                emit_projT(wk, ht, kt_loc, bk)
                emit_v(wv, ht, bvb, vt_loc, 4)

                # -- exchange K/V shards with pair partner --
                ci, co = cc_in_d[l_], cc_out_d[l_]
                nc.sync.dma_start(out=ci[:, 0:KC],
                                  in_=kt_loc.rearrange("p t s -> p (t s)"))
                nc.sync.dma_start(out=ci[:, KC:KC + VC],
                                  in_=vt_loc.rearrange("p a h d -> p (a h d)"))
                nc.gpsimd.collective_compute(
                    "AllGather", ALU.bypass, replica_groups=PAIRS,
                    ins=[ci[:]], outs=[co[:]])

                # -- self Q (overlaps the collective) --
                wq = load_w(w_slice(w_d["qs"], l_))
                bq = load_bias_pp(b_d["qs"], l_)
                qt = act.tile([128, NT, SQ], BF16, tag="qt")
                emit_projT(wq, ht, qt, bq)

                # -- cross K/V from full static memories (overlaps too) --
                kvt = {}
                for a, mem in (("l", lmem), ("r", rmem)):
                    wk_ = load_w(w_slice(w_d["k" + a], l_))
                    wv_ = load_w(w_slice(w_d["v" + a], l_))
                    bk_ = load_bias_pp(b_d["k" + a], l_)
                    bvb_ = load_bvb(a, l_)
                    kt_t = kv.tile([128, NT, M], BF16, tag="kt")
                    vt = kv.tile([128, TT, H, DK + 1], BF16, tag="vt")
                    emit_projT(wk_, mem, kt_t, bk_, ncols=M)
                    emit_v(wv_, mem, bvb_, vt, TT)
                    kvt[a] = (kt_t, vt)

                # -- receive gathered self K/V --
                kt_s = kv.tile([128, NT, 2 * SQ], BF16, tag="kt")
                vt_s = kv.tile([128, TT, H, DK + 1], BF16, tag="vt")
                for rk in range(2):
                    nc.sync.dma_start(
                        out=kt_s[:, :, rk * SQ:(rk + 1) * SQ],
                        in_=co[rk, :, 0:KC].rearrange("p (t s) -> p t s", t=NT))
                    nc.sync.dma_start(
                        out=vt_s[:, rk * 4:(rk + 1) * 4, :, :],
                        in_=co[rk, :, KC:KC + VC].rearrange(
                            "p (a h d) -> p a h d", a=4, h=H))

                at = act.tile([128, NT, SQ], BF16, tag="ha")
                emit_attn(True, qt, kt_s, vt_s, at, dbg_attn=(dbg and l_ == 0))
                if dbg and l_ == 0:
                    nc.sync.dma_start(out=dbg_d["kt"][:], in_=kt_s)
                    nc.sync.dma_start(out=dbg_d["vt"][:], in_=vt_s)
                    nc.sync.dma_start(out=dbg_d["qt"][:], in_=qt)
                    nc.sync.dma_start(out=dbg_d["at"][:], in_=at)
                wo = load_w(w_slice(w_d["os"], l_))
                bo = load_bias_pp(b_d["os"], l_)
                emit_resid(wo, at, bo)
                if dbg and l_ == 0:
                    x1c = act.tile([128, NT, SQ], F32, tag="qt")
                    nc.vector.tensor_copy(out=x1c, in_=xt)
                    nc.sync.dma_start(out=dbg_d["x1"][:], in_=x1c)

                for si, a in ((1, "l"), (2, "r")):
                    ht = act.tile([128, NT, SQ], BF16, tag="ha")
                    emit_ln(xt, lns[:, si, :], lnb[:, si, :], ht)
                    wq_ = load_w(w_slice(w_d["q" + a], l_))
                    bq_ = load_bias_pp(b_d["q" + a], l_)
                    qt = act.tile([128, NT, SQ], BF16, tag="qt")
                    emit_projT(wq_, ht, qt, bq_)
                    at = act.tile([128, NT, SQ], BF16, tag="ha")
                    emit_attn(False, qt, kvt[a][0], kvt[a][1], at)
                    if dbg and l_ == 0 and a == "l":
                        nc.sync.dma_start(out=dbg_d["ktl"][:], in_=kvt[a][0])
                        nc.sync.dma_start(out=dbg_d["vtl"][:], in_=kvt[a][1])
                        nc.sync.dma_start(out=dbg_d["qtl"][:], in_=qt)
                        nc.sync.dma_start(out=dbg_d["atl"][:], in_=at)
                    if dbg and l_ == 0 and a == "r":
                        nc.sync.dma_start(out=dbg_d["atr"][:], in_=at)
                    wo_ = load_w(w_slice(w_d["o" + a], l_))
                    bo_ = load_bias_pp(b_d["o" + a], l_)
                    emit_resid(wo_, at, bo_)
                    if dbg and l_ == 0:
                        xc = act.tile([128, NT, SQ], F32, tag="qt")
                        nc.vector.tensor_copy(out=xc, in_=xt)
                        nc.sync.dma_start(
                            out=dbg_d["x2" if a == "l" else "x3"][:], in_=xc)

                # ---- FFN ----
                ht = act.tile([128, NT, SQ], BF16, tag="ha")
                emit_ln(xt, lns[:, 3, :], lnb[:, 3, :], ht)
                b1 = lparam.tile([128, DFF // 128], F32, tag="b1")
                nc.sync.dma_start(out=b1, in_=b_d["1"][l_])
                b2 = load_bias_pp(b_d["2"], l_)
                for qr in range(4):
                    h1 = act.tile([128, 4, SQ], BF16, tag="h1")
                    w1 = load_w(w_slice(w_d["1"], l_, q=qr))
                    w2 = load_w(
                        w_d["2"][l_].rearrange("(t p) n -> p t n", p=128)
                        [:, qr * 4:(qr + 1) * 4, :])
                    for dt_ in range(4):
                        ps = ps_mm.tile([128, 512], F32, tag="mm")
                        for k in range(NT):
                            nc.tensor.matmul(
                                ps, w1[:, k, dt_ * 128:(dt_ + 1) * 128],
                                ht[:, k, :],
                                start=(k == 0), stop=(k == NT - 1))
                        nc.scalar.activation(
                            h1[:, dt_, :], ps, AF.Gelu_apprx_tanh,
                            bias=b1[:, qr * 4 + dt_:qr * 4 + dt_ + 1])
                    for m_ in range(NT):
                        ps = ps_mm.tile([128, 512], F32, tag="mm")
                        for dt_ in range(4):
                            nc.tensor.matmul(
                                ps, w2[:, dt_, m_ * 128:(m_ + 1) * 128],
                                h1[:, dt_, :],
                                start=(dt_ == 0), stop=(dt_ == 3))
                        bsl = b2[:, m_:m_ + 1] if qr == 0 else zcol
                        nc.vector.scalar_tensor_tensor(
                            out=xt[:, m_, :], in0=ps, scalar=bsl,
                            in1=xt[:, m_, :], op0=ALU.add, op1=ALU.add)
                if dbg and l_ == 0:
                    xc4 = act.tile([128, NT, SQ], F32, tag="qt")
                    nc.vector.tensor_copy(out=xc4, in_=xt)
                    nc.sync.dma_start(out=dbg_d["x4"][:], in_=xc4)

            # ---- final LN + output ----
            fns = lparam.tile([128, NT], F32, tag="fns")
            fnb = lparam.tile([128, NT], F32, tag="fnb")
            nc.sync.dma_start(out=fns, in_=fns_d[0])
            nc.sync.dma_start(out=fnb, in_=fnb_d[0])
            outt = act.tile([128, NT, SQ], F32, tag="qt")
            emit_ln(xt, fns, fnb, outt)
            nc.sync.dma_start(out=out_d.rearrange("(t p) s -> p t s", p=128),
                              in_=outt)

    nc.compile()
    return nc


def _prep_inputs(inputs, num_layers=L):
    """Build per-core in_maps from the full problem inputs."""
    nl = num_layers
    f32 = np.float32
    g = {k: np.asarray(v, dtype=f32) if np.asarray(v).dtype != np.bool_ else v
         for k, v in inputs.items()}

    def pp(a):  # [nl, D] -> [nl, 128, NT] per-partition layout
        return np.ascontiguousarray(
            a[:nl].reshape(nl, NT, 128).transpose(0, 2, 1))

    common = {}
    for i, a in enumerate(("s", "l", "r")):
        wqkv = g["Wqkv_self" if a == "s" else f"Wqkv_{a}"][:nl]
        bqkv = g["bqkv_self" if a == "s" else f"bqkv_{a}"][:nl]
        wo = g["Wo_self" if a == "s" else f"Wo_{a}"][:nl]
        bo = g["bo_self" if a == "s" else f"bo_{a}"][:nl]
        common[f"wq{a}"] = np.ascontiguousarray(wqkv[:, 0]).astype(ml_dtypes.bfloat16)
        common[f"wk{a}"] = np.ascontiguousarray(wqkv[:, 1]).astype(ml_dtypes.bfloat16)
        common[f"wv{a}"] = np.ascontiguousarray(wqkv[:, 2]).astype(ml_dtypes.bfloat16)
        common[f"wo{a}"] = np.ascontiguousarray(wo).astype(ml_dtypes.bfloat16)
        common[f"bq{a}"] = pp(bqkv[:, 0])
        common[f"bk{a}"] = pp(bqkv[:, 1])
        common[f"bv{a}"] = np.ascontiguousarray(bqkv[:, 2]).reshape(nl, 1, D)
        common[f"bo{a}"] = pp(bo)
    common["w1"] = np.ascontiguousarray(g["W1"][:nl]).astype(ml_dtypes.bfloat16)
    common["w2"] = np.ascontiguousarray(g["W2"][:nl]).astype(ml_dtypes.bfloat16)
    common["b1"] = np.ascontiguousarray(
        g["b1"][:nl].reshape(nl, DFF // 128, 128).transpose(0, 2, 1))
    common["b2"] = pp(g["b2"][:nl])
    common["lns"] = np.ascontiguousarray(
        g["ln_scale"][:nl].reshape(nl, 4, NT, 128).transpose(0, 1, 3, 2))
    common["lnb"] = np.ascontiguousarray(
        g["ln_bias"][:nl].reshape(nl, 4, NT, 128).transpose(0, 1, 3, 2))
    common["fns"] = g["fnorm_scale"].reshape(1, NT, 128).transpose(0, 2, 1).copy()
    common["fnb"] = g["fnorm_bias"].reshape(1, NT, 128).transpose(0, 2, 1).copy()

    # Masks: score layout is s^T[k, q]. m_e gates even-token k-tiles on the
    # diagonal slot (k_tok <= q_tok <=> kk <= qq for both ranks); m_o gates
    # odd-token k-tiles (strict for rank 0, non-strict for rank 1).
    tri = np.tril(np.ones((128, 128), f32)).T          # [kk, qq]: kk <= qq
    stri = np.tril(np.ones((128, 128), f32), -1).T     # kk < qq

    in_maps = []
    for c in range(8):
        b, r = c // 2, c % 2
        m = dict(common)
        m["xT"] = np.ascontiguousarray(g["tgt_emb"][b].T[:, r::2])
        m["lmemT"] = np.ascontiguousarray(g["l_mem_emb"][b].T).astype(ml_dtypes.bfloat16)
        m["rmemT"] = np.ascontiguousarray(g["r_mem_emb"][b].T).astype(ml_dtypes.bfloat16)
        m["m_e"] = tri.astype(ml_dtypes.bfloat16)
        m["m_o"] = (stri if r == 0 else tri).astype(ml_dtypes.bfloat16)
        in_maps.append(m)
    return in_maps


def run(inputs, num_layers=L, trace=False, tmpdir=None):
    key = num_layers
    if key not in _cache:
        _cache[key] = build_program(num_layers)
    nc = _cache[key]
    in_maps = _prep_inputs(inputs, num_layers)
    res = run_bass_kernel_spmd(nc, in_maps, core_ids=list(range(8)),
                               trace=trace, tmpdir=tmpdir)
    out = np.empty((B, S, D), dtype=np.float32)
    for c in range(8):
        b, r = c // 2, c % 2
        out[b, r::2, :] = res.results[c]["out"].T
    return out, res


def kernel(**inputs):
    out, _ = run(inputs)
    return out.astype(np.float32)
